# revision 3
# baseline (speedup 1.0000x reference)
"""Trainium2 kernel for nn_EdgeEmbeddingBlock (gnn_message_passing).

Computes, per edge b:
    rf  = radial_feats @ W.T + b               [E, 8]
    sa  = node_attrs[edge_index[0]]            [E, 4]
    out = einsum('bi,bk,bj->bkij', rf, sa, ea) [E, 4, 8, 16]
returns (out, out) — the reference returns the identical einsum twice.

Sharding: edges split evenly across 8 NeuronCores. The tiny linear
(262144x8 @ 8x8) and the sender-gather are folded into host-side input
sharding; each core streams its 32768-edge shard through a 512x
outer-product expansion (4 MiB of input -> 64 MiB of output per core),
which is where all the memory traffic is.

Device layout per core: edge e -> partition p = e // 256, tile t = e % 256.
Each partition's 256 edges are contiguous in DRAM, so all loads/stores are
big contiguous-per-partition DMAs. Compute is two broadcast-AP
tensor_tensor multiplies per batch of 16 tiles:
    tmp[p,t,i,j]  = rf[p,t,i] * ea[p,t,j]
    out[p,t,k,ij] = sa[p,t,k] * tmp[p,t,ij]
"""
import os
import sys

if "/opt/trn_rl_repo" not in sys.path:
    sys.path.insert(0, "/opt/trn_rl_repo")

import numpy as np

P = 128
N_CORES = 8
E = 262144
E_CORE = E // N_CORES          # 32768
N_T = E_CORE // P              # 256 tiles per core
T = 16                         # tiles per batch
N_B = N_T // T                 # 16 batches
NMAX, K, J = 8, 4, 16
V = K * NMAX * J               # 512 output values per edge

_NC = None                     # cached Bass module
LAST_RESULTS = None            # BassKernelResults of the last run (for test.py)


def _build_nc():
    import concourse.bacc as bacc
    import concourse.mybir as mybir
    from concourse.tile import TileContext

    F32 = mybir.dt.float32
    nc = bacc.Bacc()
    rf_d = nc.dram_tensor("rf", [E_CORE, NMAX], F32, kind="ExternalInput")
    sa_d = nc.dram_tensor("sa", [E_CORE, K], F32, kind="ExternalInput")
    ea_d = nc.dram_tensor("ea", [E_CORE, J], F32, kind="ExternalInput")
    out_d = nc.dram_tensor("out", [E_CORE, V], F32, kind="ExternalOutput")

    rf_v = rf_d.rearrange("(p t) f -> p (t f)", p=P)
    sa_v = sa_d.rearrange("(p t) f -> p (t f)", p=P)
    ea_v = ea_d.rearrange("(p t) f -> p (t f)", p=P)
    out_v = out_d.rearrange("(p t) v -> p (t v)", p=P)

    with TileContext(nc) as tc:
        with (
            tc.tile_pool(name="in_pool", bufs=1) as in_pool,
            tc.tile_pool(name="tmp_pool", bufs=2) as tmp_pool,
            tc.tile_pool(name="out_pool", bufs=3) as out_pool,
        ):
            rf_t = in_pool.tile([P, N_T * NMAX], F32, tag="rf")
            sa_t = in_pool.tile([P, N_T * K], F32, tag="sa")
            ea_t = in_pool.tile([P, N_T * J], F32, tag="ea")
            nc.sync.dma_start(out=rf_t[:], in_=rf_v[:, :])
            nc.sync.dma_start(out=sa_t[:], in_=sa_v[:, :])
            nc.sync.dma_start(out=ea_t[:], in_=ea_v[:, :])

            for b in range(N_B):
                tmp_t = tmp_pool.tile([P, T * NMAX * J], F32, tag="tmp")
                out_t = out_pool.tile([P, T * V], F32, tag="out")

                rf_s = rf_t[:, b * T * NMAX:(b + 1) * T * NMAX]
                sa_s = sa_t[:, b * T * K:(b + 1) * T * K]
                ea_s = ea_t[:, b * T * J:(b + 1) * T * J]

                rf_b = (rf_s.rearrange("p (t i) -> p t i", i=NMAX)
                        .unsqueeze(3).broadcast_to([P, T, NMAX, J]))
                ea_b = (ea_s.rearrange("p (t j) -> p t j", j=J)
                        .unsqueeze(2).broadcast_to([P, T, NMAX, J]))
                tmp_view = tmp_t[:].rearrange("p (t i j) -> p t i j", i=NMAX, j=J)
                nc.vector.tensor_tensor(out=tmp_view, in0=rf_b, in1=ea_b,
                                        op=mybir.AluOpType.mult)

                sa_b = (sa_s.rearrange("p (t k) -> p t k", k=K)
                        .unsqueeze(3).broadcast_to([P, T, K, NMAX * J]))
                tmp_b = (tmp_t[:].rearrange("p (t ij) -> p t ij", ij=NMAX * J)
                         .unsqueeze(2).broadcast_to([P, T, K, NMAX * J]))
                out_view = out_t[:].rearrange("p (t k ij) -> p t k ij",
                                              k=K, ij=NMAX * J)
                nc.vector.tensor_tensor(out=out_view, in0=sa_b, in1=tmp_b,
                                        op=mybir.AluOpType.mult)

                nc.sync.dma_start(out=out_v[:, b * T * V:(b + 1) * T * V],
                                  in_=out_t[:])
    nc.finalize()
    return nc


def kernel(edge_index, radial_feats, edge_attrs, node_attrs, W, b):
    global _NC, LAST_RESULTS
    from concourse.bass_utils import run_bass_kernel_spmd

    edge_index = np.asarray(edge_index)
    radial_feats = np.ascontiguousarray(np.asarray(radial_feats, dtype=np.float32))
    edge_attrs = np.ascontiguousarray(np.asarray(edge_attrs, dtype=np.float32))
    node_attrs = np.ascontiguousarray(np.asarray(node_attrs, dtype=np.float32))
    W = np.asarray(W, dtype=np.float32)
    bias = np.asarray(b, dtype=np.float32)

    # Host-side sharding prep: fold the 8x8 linear and the sender-gather
    # into the per-core input shards.
    sender = edge_index[0].astype(np.int64)
    rf = radial_feats @ W.T + bias               # [E, 8] f32
    sa = node_attrs[sender]                      # [E, 4] f32

    if _NC is None:
        _NC = _build_nc()

    in_maps = []
    for c in range(N_CORES):
        lo, hi = c * E_CORE, (c + 1) * E_CORE
        in_maps.append({
            "rf": np.ascontiguousarray(rf[lo:hi]),
            "sa": np.ascontiguousarray(sa[lo:hi]),
            "ea": np.ascontiguousarray(edge_attrs[lo:hi]),
        })

    trace = bool(os.environ.get("KERNEL_TRACE"))
    res = run_bass_kernel_spmd(_NC, in_maps, list(range(N_CORES)), trace=trace)
    LAST_RESULTS = res

    out = np.concatenate([np.asarray(res.results[c]["out"])
                          for c in range(N_CORES)], axis=0)
    out = out.reshape(E, K, NMAX, J)
    return (out, out)


# revision 4
# speedup vs baseline: 1.3194x; 1.3194x over previous
"""Trainium2 kernel for nn_EdgeEmbeddingBlock (gnn_message_passing).

Computes, per edge b:
    rf  = radial_feats @ W.T + b               [E, 8]
    sa  = node_attrs[edge_index[0]]            [E, 4]
    out = einsum('bi,bk,bj->bkij', rf, sa, ea) [E, 4, 8, 16]
returns (out, out) — the reference returns the identical einsum twice.

Sharding: edges split evenly across 8 NeuronCores. The tiny linear
(262144x8 @ 8x8) and the sender-gather are folded into host-side input
sharding (they are 0.7% of the bytes); each core then streams its
32768-edge shard through a 512x outer-product expansion (3.5 MiB in ->
64 MiB out per core), which is where all the memory traffic is. The
kernel is HBM-write-bound: 64 MiB / ~358 GB/s ~= 188 us per core.

Device layout per core: edge e -> partition p = e // 256, tile t = e % 256,
so every partition's edges are contiguous in DRAM and all DMAs move large
contiguous per-partition chunks. Inputs rf|sa|ea are host-packed into one
[E_CORE, 28] tensor: one input DMA stream instead of three.

Compute per batch of T=8 tiles (1024 edges) is two broadcast-AP
tensor_tensor multiplies on the vector engine:
    tmp[p,t,i,j]  = rf[p,t,i] * ea[p,t,j]      (in0 step-0 over j)
    out[p,t,k,ij] = sa[p,t,k] * tmp[p,t,ij]    (in0 step-0 over ij)
The input preload is chunked (2,6,24 batches) so the first store issues
~8 us into the kernel while the bulk of the input load overlaps the
store stream.
"""
import os
import sys

if "/opt/trn_rl_repo" not in sys.path:
    sys.path.insert(0, "/opt/trn_rl_repo")

import numpy as np

P = 128
N_CORES = 8
E = 262144
E_CORE = E // N_CORES          # 32768
N_T = E_CORE // P              # 256 tiles per core
T = 8                          # tiles per batch
N_B = N_T // T                 # 32 batches
CHUNKS = (2, 6, 24)            # input preload chunks, in batches
NMAX, K, J = 8, 4, 16
F = NMAX + K + J               # 28 packed input features per edge
V = K * NMAX * J               # 512 output values per edge

_NC = None                     # cached Bass module
LAST_RESULTS = None            # BassKernelResults of the last run (for test.py)


def _build_nc():
    import concourse.bacc as bacc
    import concourse.mybir as mybir
    from concourse.tile import TileContext

    F32 = mybir.dt.float32
    nc = bacc.Bacc()
    pk_d = nc.dram_tensor("pk", [E_CORE, F], F32, kind="ExternalInput")
    out_d = nc.dram_tensor("out", [E_CORE, V], F32, kind="ExternalOutput")

    pk_v = pk_d.rearrange("(p t) f -> p (t f)", p=P)
    out_v = out_d.rearrange("(p t) v -> p (t v)", p=P)

    with TileContext(nc) as tc:
        with (
            tc.tile_pool(name="in_pool", bufs=1) as in_pool,
            tc.tile_pool(name="tmp_pool", bufs=2) as tmp_pool,
            tc.tile_pool(name="out_pool", bufs=3) as out_pool,
        ):
            pk_all = in_pool.tile([P, N_T * F], F32, tag="pk")
            b0 = 0
            for csz in CHUNKS:
                lo, hi = b0 * T * F, (b0 + csz) * T * F
                nc.sync.dma_start(out=pk_all[:, lo:hi], in_=pk_v[:, lo:hi])
                b0 += csz
            assert b0 == N_B

            for b in range(N_B):
                tmp_t = tmp_pool.tile([P, T * NMAX * J], F32, tag="tmp")
                out_t = out_pool.tile([P, T * V], F32, tag="out")

                pk = (pk_all[:, b * T * F:(b + 1) * T * F]
                      .rearrange("p (t f) -> p t f", f=F))
                rf_b = pk[:, :, 0:NMAX].unsqueeze(3).broadcast_to([P, T, NMAX, J])
                ea_b = pk[:, :, NMAX + K:F].unsqueeze(2).broadcast_to([P, T, NMAX, J])
                tmp_view = tmp_t[:].rearrange("p (t i j) -> p t i j", i=NMAX, j=J)
                nc.vector.tensor_tensor(out=tmp_view, in0=rf_b, in1=ea_b,
                                        op=mybir.AluOpType.mult)

                sa_b = (pk[:, :, NMAX:NMAX + K]
                        .unsqueeze(3).broadcast_to([P, T, K, NMAX * J]))
                tmp_b = (tmp_t[:].rearrange("p (t ij) -> p t ij", ij=NMAX * J)
                         .unsqueeze(2).broadcast_to([P, T, K, NMAX * J]))
                out_view = out_t[:].rearrange("p (t k ij) -> p t k ij",
                                              k=K, ij=NMAX * J)
                nc.vector.tensor_tensor(out=out_view, in0=sa_b, in1=tmp_b,
                                        op=mybir.AluOpType.mult)

                nc.sync.dma_start(out=out_v[:, b * T * V:(b + 1) * T * V],
                                  in_=out_t[:])
    nc.finalize()
    return nc


def kernel(edge_index, radial_feats, edge_attrs, node_attrs, W, b):
    global _NC, LAST_RESULTS
    from concourse.bass_utils import run_bass_kernel_spmd

    edge_index = np.asarray(edge_index)
    radial_feats = np.asarray(radial_feats, dtype=np.float32)
    edge_attrs = np.asarray(edge_attrs, dtype=np.float32)
    node_attrs = np.asarray(node_attrs, dtype=np.float32)
    W = np.asarray(W, dtype=np.float32)
    bias = np.asarray(b, dtype=np.float32)

    # Host-side sharding prep: fold the 8x8 linear and the sender-gather
    # into the per-core packed input shards.
    sender = edge_index[0].astype(np.int64)
    rf = radial_feats @ W.T + bias               # [E, 8]
    sa = node_attrs[sender]                      # [E, 4]
    pk = np.concatenate([rf, sa, edge_attrs], axis=1)  # [E, 28]

    if _NC is None:
        _NC = _build_nc()

    in_maps = [{"pk": np.ascontiguousarray(pk[c * E_CORE:(c + 1) * E_CORE])}
               for c in range(N_CORES)]

    trace = bool(os.environ.get("KERNEL_TRACE"))
    res = run_bass_kernel_spmd(_NC, in_maps, list(range(N_CORES)), trace=trace)
    LAST_RESULTS = res

    out = np.concatenate([np.asarray(res.results[c]["out"])
                          for c in range(N_CORES)], axis=0)
    out = out.reshape(E, K, NMAX, J)
    return (out, out)


# revision 6
# speedup vs baseline: 1.4207x; 1.0767x over previous
"""Trainium2 kernel for nn_EdgeEmbeddingBlock (gnn_message_passing).

Computes, per edge b:
    rf  = radial_feats @ W.T + b               [E, 8]
    sa  = node_attrs[edge_index[0]]            [E, 4]
    out = einsum('bi,bk,bj->bkij', rf, sa, ea) [E, 4, 8, 16]
returns (out, out) — the reference returns the identical einsum twice.

Sharding: edges split evenly across 8 NeuronCores. The tiny linear
(262144x8 @ 8x8) and the sender-gather are folded into host-side input
sharding (they are 0.7% of the bytes); each core then streams its
32768-edge shard through a 512x outer-product expansion (3.5 MiB in ->
64 MiB out per core), which is where all the memory traffic is. The
kernel is HBM-write-bound: 64 MiB / ~358 GB/s ~= 188 us per core.

Device layout per core: edge e -> partition p = e // 256, tile t = e % 256,
so every partition's edges are contiguous in DRAM and all DMAs move large
contiguous per-partition chunks. Inputs rf|sa|ea are host-packed into one
[E_CORE, 28] tensor: one input DMA stream instead of three.

Compute per batch of T=8 tiles (1024 edges) is two broadcast-AP
tensor_tensor multiplies on the vector engine:
    tmp[p,t,i,j]  = rf[p,t,i] * ea[p,t,j]      (in0 step-0 over j)
    out[p,t,k,ij] = sa[p,t,k] * tmp[p,t,ij]    (in0 step-0 over ij)
The input preload is chunked (2,6,24 batches) so the first store issues
~8 us into the kernel while the bulk of the input load overlaps the
store stream.
"""
import os
import sys

if "/opt/trn_rl_repo" not in sys.path:
    sys.path.insert(0, "/opt/trn_rl_repo")

import numpy as np

P = 128
N_CORES = 8
E = 262144
E_CORE = E // N_CORES          # 32768
N_T = E_CORE // P              # 256 tiles per core
# Batch schedule in tiles: small warm-up batches shrink the pipeline fill
# (first store issues ~2 us after the first 28 KB input chunk lands),
# then steady-state batches of 8 tiles (1024 edges, 2 MiB stores).
SCHEDULE = (2, 2, 4) + (8,) * 31
CHUNKS = (2, 6, 56, 192)       # input preload chunk sizes, in tiles
OUT_BUFS = 8                   # store slots in flight (HW-A/B'd optimum)
TMP_BUFS = 2
NMAX, K, J = 8, 4, 16
F = NMAX + K + J               # 28 packed input features per edge
V = K * NMAX * J               # 512 output values per edge

_NC = None                     # cached Bass module
LAST_RESULTS = None            # BassKernelResults of the last run (for test.py)


def _build_nc():
    import concourse.bacc as bacc
    import concourse.mybir as mybir
    from concourse.tile import TileContext

    F32 = mybir.dt.float32
    nc = bacc.Bacc()
    pk_d = nc.dram_tensor("pk", [E_CORE, F], F32, kind="ExternalInput")
    out_d = nc.dram_tensor("out", [E_CORE, V], F32, kind="ExternalOutput")

    pk_v = pk_d.rearrange("(p t) f -> p (t f)", p=P)
    out_v = out_d.rearrange("(p t) v -> p (t v)", p=P)

    with TileContext(nc) as tc:
        with (
            tc.tile_pool(name="in_pool", bufs=1) as in_pool,
            tc.tile_pool(name="tmp_pool", bufs=TMP_BUFS) as tmp_pool,
            tc.tile_pool(name="out_pool", bufs=OUT_BUFS) as out_pool,
        ):
            pk_all = in_pool.tile([P, N_T * F], F32, tag="pk")
            t0 = 0
            for csz in CHUNKS:
                nc.sync.dma_start(out=pk_all[:, t0 * F:(t0 + csz) * F],
                                  in_=pk_v[:, t0 * F:(t0 + csz) * F])
                t0 += csz
            assert t0 == N_T

            t0 = 0
            for bt in SCHEDULE:
                tmp_t = tmp_pool.tile([P, bt * NMAX * J], F32, tag="tmp")
                out_t = out_pool.tile([P, bt * V], F32, tag="out")

                pk = (pk_all[:, t0 * F:(t0 + bt) * F]
                      .rearrange("p (t f) -> p t f", f=F))
                rf_b = pk[:, :, 0:NMAX].unsqueeze(3).broadcast_to([P, bt, NMAX, J])
                ea_b = pk[:, :, NMAX + K:F].unsqueeze(2).broadcast_to([P, bt, NMAX, J])
                tmp_view = tmp_t[:].rearrange("p (t i j) -> p t i j", i=NMAX, j=J)
                nc.vector.tensor_tensor(out=tmp_view, in0=rf_b, in1=ea_b,
                                        op=mybir.AluOpType.mult)

                sa_b = (pk[:, :, NMAX:NMAX + K]
                        .unsqueeze(3).broadcast_to([P, bt, K, NMAX * J]))
                tmp_b = (tmp_t[:].rearrange("p (t ij) -> p t ij", ij=NMAX * J)
                         .unsqueeze(2).broadcast_to([P, bt, K, NMAX * J]))
                out_view = out_t[:].rearrange("p (t k ij) -> p t k ij",
                                              k=K, ij=NMAX * J)
                nc.vector.tensor_tensor(out=out_view, in0=sa_b, in1=tmp_b,
                                        op=mybir.AluOpType.mult)

                nc.sync.dma_start(out=out_v[:, t0 * V:(t0 + bt) * V],
                                  in_=out_t[:])
                t0 += bt
            assert t0 == N_T
    nc.finalize()
    return nc


def kernel(edge_index, radial_feats, edge_attrs, node_attrs, W, b):
    global _NC, LAST_RESULTS
    from concourse.bass_utils import run_bass_kernel_spmd

    edge_index = np.asarray(edge_index)
    radial_feats = np.asarray(radial_feats, dtype=np.float32)
    edge_attrs = np.asarray(edge_attrs, dtype=np.float32)
    node_attrs = np.asarray(node_attrs, dtype=np.float32)
    W = np.asarray(W, dtype=np.float32)
    bias = np.asarray(b, dtype=np.float32)

    # Host-side sharding prep: fold the 8x8 linear and the sender-gather
    # into the per-core packed input shards.
    sender = edge_index[0].astype(np.int64)
    rf = radial_feats @ W.T + bias               # [E, 8]
    sa = node_attrs[sender]                      # [E, 4]
    pk = np.concatenate([rf, sa, edge_attrs], axis=1)  # [E, 28]

    if _NC is None:
        _NC = _build_nc()

    in_maps = [{"pk": np.ascontiguousarray(pk[c * E_CORE:(c + 1) * E_CORE])}
               for c in range(N_CORES)]

    trace = bool(os.environ.get("KERNEL_TRACE"))
    res = run_bass_kernel_spmd(_NC, in_maps, list(range(N_CORES)), trace=trace)
    LAST_RESULTS = res

    out = np.concatenate([np.asarray(res.results[c]["out"])
                          for c in range(N_CORES)], axis=0)
    out = out.reshape(E, K, NMAX, J)
    return (out, out)
